# revision 47
# baseline (speedup 1.0000x reference)
"""Trainium2 Bass kernel for nn_BidPrefix (segment_reduce).

Reference semantics, per row r (B=65536 rows, S=512 cols):
    cp[k]    = prod(x[r, 0:k])                  (exclusive prefix product)
    survival = cp[bid]
    rate     = cp[mp] - cp[mp+1], or EPS when mp == 0
returned as (survival [B,1] f32, rate_last [B,1] f32).

Design: exact fp32 inclusive cumprod via DVE tensor_tensor_scan
(op0=mult, op1=bypass; one 512-long recurrence per row-group, written
in place over the x tile), then a per-row 3-element extraction with one
GPSIMD ap_gather per chunk:
    survival = cpi[bid-1]   (bid==0 -> 1, fixed up)
    g1       = cpi[mp-1]    (mp==0 handled by the EPS fixup)
    g2       = cpi[mp]
    rate     = mp==0 ? EPS : g1 - g2
ap_gather applies, for each 16-partition GPSIMD core, the index list
stored across its 16 partitions (slot s of partition p = flat position
q = s*16 + p%16) to ALL 16 channels; row p's own values land at
out[p, s*16 + p%16] and are pulled out with a static one-hot mask
(mult + segmented reduce).  Indices are pre-offset by glocal*512 so a
single gather covers a whole multi-group chunk.

Schedule: the three ~47us engine streams (DMA-in, DVE scans+extracts,
Pool gathers) are software-pipelined: a chunk's extract is deferred
DEFER chunks so the in-order DVE never stalls on the Pool gather it
consumes.  Chunk sizes taper (4..4,2..2 groups) to shorten pipeline
fill and drain; index prep casts run on the (otherwise idle
until the first gather) Pool engine; fixups are predicated copies,
split so only the last chunk's sliver trails the final gather; both
outputs interleave into one DRAM tensor so the tail pays a single
DMA chain.

Row mapping r = p*64 + k keeps every DMA contiguous per partition:
x chunks 4-16KB, bid_info 512B, output 512B.

Sharding: pure data parallel over the batch axis, B/8 = 8192 rows per
NeuronCore, same NEFF on all 8 cores (SPMD), outputs concatenated.
"""

import numpy as np

import concourse.bacc as bacc
import concourse.mybir as mybir
from concourse.tile import TileContext
from concourse.bass_utils import run_bass_kernel_spmd

f32 = mybir.dt.float32
i32 = mybir.dt.int32
i16 = mybir.dt.int16
Alu = mybir.AluOpType
Ax = mybir.AxisListType

N_CORES = 8
B, S = 65536, 512
ROWS = B // N_CORES          # 8192 rows per core
K = ROWS // 128              # 64 row-groups per partition
# Chunk sizes must be EVEN: a chunk's int16 index slice is 3*gc slots per
# partition, and the hardware ap_gather ucode needs 4-byte-aligned index
# slices (odd gc -> 18-byte offsets -> silently wrong gathers on HW, even
# though the interpreter/cost model accept it).
CHUNKS = [4] * 11 + [2] * 10                # groups per chunk, sum = 64
EBATCH = 1                                 # chunks per extract batch
DEFER = 2                                  # batches between gather and extract
SPLIT_HEAD = 0                             # leading chunks with per-group DMA
assert sum(CHUNKS) == K
assert all(gc % 2 == 0 for gc in CHUNKS), "odd chunks break ap_gather on HW"
EPS = 1e-7


def build_bass():
    nc = bacc.Bacc()

    x = nc.dram_tensor("x", [ROWS, S], f32, kind="ExternalInput")
    bid_info = nc.dram_tensor("bid_info", [ROWS, 2], i32, kind="ExternalInput")
    out2 = nc.dram_tensor("out2", [ROWS, 2], f32, kind="ExternalOutput")

    # row r = p*64 + k  ->  every DMA contiguous per partition
    x_v = x.rearrange("(p k) s -> p (k s)", p=128)           # [128, 64*512]
    bi_v = bid_info.rearrange("(p k) c -> p (k c)", p=128)   # [128, 128]
    o2_v = out2.rearrange("(p k) c -> p (k c)", p=128)       # [128, 128]

    import contextlib

    # Ring depth per chunk-size class: deep enough to keep the DMA stream
    # gapless, within a ~120KB/partition budget split across classes.
    classes = sorted(set(CHUNKS))
    per_class_kb = 120 // len(classes)
    BUFS = {gc: max(2, min(10, per_class_kb // (2 * gc))) for gc in classes}

    with TileContext(nc) as tc:
        with contextlib.ExitStack() as stack:
            cpool = stack.enter_context(tc.tile_pool(name="const", bufs=1))
            spool = stack.enter_context(tc.tile_pool(name="small", bufs=5))
            pools = {
                gc: stack.enter_context(
                    tc.tile_pool(name=f"big{gc}", bufs=BUFS[gc]))
                for gc in classes
            }

            # ---- bid_info first (tiny; unblocks index prep), then x chunk 0
            bi_all = cpool.tile([128, 2 * K], i32, tag="bi_all")
            nc.sync.dma_start(out=bi_all[:], in_=bi_v)
            g0 = CHUNKS[0]
            xt0 = pools[g0].tile([128, g0 * S], f32, tag=f"xt{g0}")
            w0 = S if SPLIT_HEAD > 0 else g0 * S
            nc.sync.dma_start(out=xt0[:, 0:w0], in_=x_v[:, 0:w0])

            # ---- static constants + index prep, all on Pool (idle until
            # the first gather) so the DVE can start scanning immediately --
            MW = 48 * max(sum(CHUNKS[i:i + EBATCH])
                          for i in range(0, len(CHUNKS), EBATCH))
            rq = cpool.tile([128, MW], i32, tag="rq")          # q%16 per slot
            nc.gpsimd.iota(rq[:], pattern=[[0, MW // 16], [1, 16]], base=0,
                           channel_multiplier=0)
            pp = cpool.tile([128, 1], i32, tag="pp")           # partition idx
            nc.gpsimd.iota(pp[:], pattern=[[1, 1]], base=0,
                           channel_multiplier=1)
            offs32 = cpool.tile([128, 3 * K], i32, tag="offs32")  # glocal*512
            gs = 0
            for gc in CHUNKS:
                nc.gpsimd.iota(offs32[:, 3 * gs:3 * (gs + gc)],
                               pattern=[[512, gc], [0, 3]], base=0,
                               channel_multiplier=0)
                gs += gc

            # Pool handles the casts (proven gpsimd ops); the
            # TensorScalarPtr-encoded arithmetic must stay on DVE (walrus
            # rejects that encoding on Pool).
            pm16 = cpool.tile([128, 1], i32, tag="pm16")
            nc.vector.tensor_scalar(out=pm16[:], in0=pp[:], scalar1=15,
                                    scalar2=None, op0=Alu.bitwise_and)
            pm16f = cpool.tile([128, 1], f32, tag="pm16f")
            nc.gpsimd.tensor_copy(out=pm16f[:], in_=pm16[:])
            rqf = cpool.tile([128, MW], f32, tag="rqf")
            nc.gpsimd.tensor_copy(out=rqf[:], in_=rq[:])
            # one-hot diag mask M[p, s*16+r] = (r == p%16), periodic in s
            m384 = cpool.tile([128, MW], f32, tag="m384")
            nc.vector.tensor_scalar(out=m384[:], in0=rqf[:], scalar1=pm16f[:],
                                    scalar2=None, op0=Alu.is_equal)
            offsf = cpool.tile([128, 3 * K], f32, tag="offsf")
            nc.gpsimd.tensor_copy(out=offsf[:], in_=offs32[:])

            bif = cpool.tile([128, 2 * K], f32, tag="bif")
            nc.gpsimd.tensor_copy(out=bif[:], in_=bi_all[:])
            bif3 = bif[:].rearrange("p (k c) -> p k c", c=2)
            mpf = bif3[:, :, 0]     # [128, 64] market price (strided)
            bidf = bif3[:, :, 1]    # [128, 64] bid

            idxf = cpool.tile([128, 3 * K], f32, tag="idxf")
            idx3 = idxf[:].rearrange("p (k j) -> p k j", j=3)
            # j=0: bid-1, j=1: mp-1, j=2: mp   (clamped at 0; fixups later)
            nc.vector.tensor_scalar(out=idx3[:, :, 0], in0=bidf, scalar1=-1.0,
                                    scalar2=0.0, op0=Alu.add, op1=Alu.max)
            nc.vector.tensor_scalar(out=idx3[:, :, 1], in0=mpf, scalar1=-1.0,
                                    scalar2=0.0, op0=Alu.add, op1=Alu.max)
            nc.gpsimd.tensor_copy(out=idx3[:, :, 2], in_=mpf)
            nc.vector.tensor_tensor(out=idxf[:], in0=idxf[:], in1=offsf[:],
                                    op=Alu.add)
            idx16 = cpool.tile([128, 3 * K], i16, tag="idx16")
            nc.gpsimd.tensor_copy(out=idx16[:], in_=idxf[:])

            # masks + fill constants for the bid==0 / mp==0 fixups
            # (CopyPredicated wants an integer mask dtype)
            mb = cpool.tile([128, K], i32, tag="mb")
            nc.vector.tensor_scalar(out=mb[:], in0=bidf, scalar1=0.0,
                                    scalar2=None, op0=Alu.is_equal)
            mm = cpool.tile([128, K], i32, tag="mm")
            nc.vector.tensor_scalar(out=mm[:], in0=mpf, scalar1=0.0,
                                    scalar2=None, op0=Alu.is_equal)
            ones = cpool.tile([128, K], f32, tag="ones")
            nc.gpsimd.memset(ones[:], 1.0)
            epsc = cpool.tile([128, K], f32, tag="epsc")
            nc.gpsimd.memset(epsc[:], EPS)

            vals = cpool.tile([128, 3 * K], f32, tag="vals")  # [p, k, j]
            ost = cpool.tile([128, 2 * K], f32, tag="ost")    # [p, k, (s,r)]
            ost3 = ost[:].rearrange("p (k c) -> p k c", c=2)
            surv = ost3[:, :, 0]    # strided [128, 64] views
            rate = ost3[:, :, 1]

            v3 = vals[:].rearrange("p (k j) -> p k j", j=3)

            def extract(bgs, bw, gat):
                msk = spool.tile([128, 48 * bw], f32, tag=f"msk{bw}")
                nc.vector.tensor_tensor(out=msk[:], in0=gat[:],
                                        in1=m384[:, 0:48 * bw], op=Alu.mult)
                m3 = msk[:].rearrange("p (s r) -> p s r", r=16)
                nc.vector.tensor_reduce(out=vals[:, 3 * bgs:3 * (bgs + bw)],
                                        in_=m3, axis=Ax.X, op=Alu.add)

            def fix_surv(lo, hi):
                sl = slice(lo, hi)
                # survival = bid==0 ? 1 : cpi[bid-1]
                nc.vector.tensor_copy(out=surv[:, sl], in_=v3[:, sl, 0])
                nc.vector.copy_predicated(out=surv[:, sl], mask=mb[:, sl],
                                          data=ones[:, sl])

            def fix_rate(lo, hi):
                sl = slice(lo, hi)
                # rate = mp==0 ? EPS : cpi[mp-1] - cpi[mp]
                nc.vector.tensor_tensor(out=rate[:, sl], in0=v3[:, sl, 1],
                                        in1=v3[:, sl, 2], op=Alu.subtract)
                nc.vector.copy_predicated(out=rate[:, sl], mask=mm[:, sl],
                                          data=epsc[:, sl])

            def fixups(lo, hi):
                fix_surv(lo, hi)
                fix_rate(lo, hi)

            # ---- main loop over chunks -----------------------------------
            # Gathers land in a shared per-batch tile; one mult+reduce
            # extracts a whole batch (fewer DVE instructions).  Extraction
            # of batch b is issued once batch b+1 is complete, so the
            # in-order DVE meets long-finished Pool gathers.
            batch_w = [sum(CHUNKS[i:i + EBATCH])
                       for i in range(0, len(CHUNKS), EBATCH)]
            pending = []    # (bgs, bw, gat) full batches awaiting extraction
            cur = None      # [bgs, bw, gat, filled] batch being filled
            gs = 0
            n_chunks = len(CHUNKS)
            for ci, gc in enumerate(CHUNKS):
                # The first chunks' DMAs are split per group so their scans
                # (and so the first gathers) chase the DMA stream instead of
                # waiting for the whole chunk to land.
                split = ci < SPLIT_HEAD
                if ci == 0:
                    xt = xt0
                else:
                    xt = pools[gc].tile([128, gc * S], f32, tag=f"xt{gc}")
                    if not split:
                        nc.sync.dma_start(out=xt[:],
                                          in_=x_v[:, gs * S:(gs + gc) * S])
                if split:
                    for g in range(gc):
                        if ci == 0 and g == 0:
                            continue  # xt0's first slice DMA issued up top
                        nc.sync.dma_start(
                            out=xt[:, g * S:(g + 1) * S],
                            in_=x_v[:, (gs + g) * S:(gs + g + 1) * S])

                for g in range(gc):
                    sl = slice(g * S, (g + 1) * S)
                    if split or g == 0:
                        # Tiny read absorbs the HWDGE queue semaphore before
                        # the TensorScalarPtr-encoded scans (that ISA encoding
                        # has too few sync-wait slots to carry it itself).
                        sink = spool.tile([128, 2], f32, tag="sink")
                        nc.vector.tensor_copy(out=sink[:, 0:1],
                                              in_=xt[:, sl.start:sl.start + 1])
                    nc.vector.tensor_tensor_scan(
                        out=xt[:, sl], data0=xt[:, sl], data1=xt[:, sl],
                        initial=1.0, op0=Alu.mult, op1=Alu.bypass)

                if cur is None:
                    bw = batch_w[ci // EBATCH]
                    gat = spool.tile([128, 48 * bw], f32, tag=f"gatb{bw}")
                    cur = [gs, bw, gat, 0]
                off = cur[3]
                nc.gpsimd.ap_gather(
                    out_ap=cur[2][:, 48 * off:48 * (off + gc)], in_ap=xt[:],
                    idxs_ap=idx16[:, 3 * gs:3 * (gs + gc)],
                    channels=128, num_elems=gc * S, d=1, num_idxs=48 * gc)
                cur[3] += gc

                if cur[3] == cur[1]:
                    pending.append((cur[0], cur[1], cur[2]))
                    cur = None
                    while len(pending) > DEFER:
                        extract(*pending.pop(0))
                if ci == n_chunks - 1:
                    # head fixups run on DVE while the last gathers run
                    while len(pending) > 1:
                        extract(*pending.pop(0))
                    fixups(0, pending[0][0])
                gs += gc

            tail_lo = pending[0][0]
            extract(*pending.pop(0))
            fix_surv(tail_lo, K)
            fix_rate(tail_lo, K)
            nc.sync.dma_start(out=o2_v, in_=ost[:])
    nc.finalize()
    return nc


_NC_CACHE = None


def _get_nc():
    global _NC_CACHE
    if _NC_CACHE is None:
        _NC_CACHE = build_bass()
    return _NC_CACHE


def kernel(x, bid_info):
    x = np.ascontiguousarray(np.asarray(x, dtype=np.float32))
    bid_info = np.ascontiguousarray(np.asarray(bid_info, dtype=np.int32))
    assert x.shape == (B, S) and bid_info.shape == (B, 2)

    nc = _get_nc()
    in_maps = [
        {
            "x": x[c * ROWS:(c + 1) * ROWS],
            "bid_info": bid_info[c * ROWS:(c + 1) * ROWS],
        }
        for c in range(N_CORES)
    ]
    res = run_bass_kernel_spmd(nc, in_maps, core_ids=list(range(N_CORES)))
    out2 = np.concatenate([r["out2"] for r in res.results], axis=0)
    return np.ascontiguousarray(out2[:, 0:1]), np.ascontiguousarray(out2[:, 1:2])


# revision 48
# speedup vs baseline: 1.0010x; 1.0010x over previous
"""Trainium2 Bass kernel for nn_BidPrefix (segment_reduce).

Reference semantics, per row r (B=65536 rows, S=512 cols):
    cp[k]    = prod(x[r, 0:k])                  (exclusive prefix product)
    survival = cp[bid]
    rate     = cp[mp] - cp[mp+1], or EPS when mp == 0
returned as (survival [B,1] f32, rate_last [B,1] f32).

Design: exact fp32 inclusive cumprod via DVE tensor_tensor_scan
(op0=mult, op1=bypass; one 512-long recurrence per row-group, written
in place over the x tile), then a per-row 3-element extraction with one
GPSIMD ap_gather per chunk:
    survival = cpi[bid-1]   (bid==0 -> 1, fixed up)
    g1       = cpi[mp-1]    (mp==0 handled by the EPS fixup)
    g2       = cpi[mp]
    rate     = mp==0 ? EPS : g1 - g2
ap_gather applies, for each 16-partition GPSIMD core, the index list
stored across its 16 partitions (slot s of partition p = flat position
q = s*16 + p%16) to ALL 16 channels; row p's own values land at
out[p, s*16 + p%16] and are pulled out with a static one-hot mask
(mult + segmented reduce).  Indices are pre-offset by glocal*512 so a
single gather covers a whole multi-group chunk.

Schedule: the three ~47us engine streams (DMA-in, DVE scans+extracts,
Pool gathers) are software-pipelined: a chunk's extract is deferred
DEFER chunks so the in-order DVE never stalls on the Pool gather it
consumes.  Chunk sizes taper (4..4,2..2 groups) to shorten pipeline
fill and drain; index prep casts run on the (otherwise idle
until the first gather) Pool engine; fixups are predicated copies,
split so only the last chunk's sliver trails the final gather; both
outputs interleave into one DRAM tensor so the tail pays a single
DMA chain.

Row mapping r = p*64 + k keeps every DMA contiguous per partition:
x chunks 4-16KB, bid_info 512B, output 512B.

Sharding: pure data parallel over the batch axis, B/8 = 8192 rows per
NeuronCore, same NEFF on all 8 cores (SPMD), outputs concatenated.
"""

import numpy as np

import concourse.bacc as bacc
import concourse.mybir as mybir
from concourse.tile import TileContext
from concourse.bass_utils import run_bass_kernel_spmd

f32 = mybir.dt.float32
i32 = mybir.dt.int32
i16 = mybir.dt.int16
Alu = mybir.AluOpType
Ax = mybir.AxisListType

N_CORES = 8
B, S = 65536, 512
ROWS = B // N_CORES          # 8192 rows per core
K = ROWS // 128              # 64 row-groups per partition
# Chunk sizes must be EVEN: a chunk's int16 index slice is 3*gc slots per
# partition, and the hardware ap_gather ucode needs 4-byte-aligned index
# slices (odd gc -> 18-byte offsets -> silently wrong gathers on HW, even
# though the interpreter/cost model accept it).
CHUNKS = [4] * 11 + [2] * 10                # groups per chunk, sum = 64
EBATCH = 1                                 # chunks per extract batch
DEFER = 2                                  # batches between gather and extract
SPLIT_HEAD = 0                             # leading chunks with per-group DMA
assert sum(CHUNKS) == K
assert all(gc % 2 == 0 for gc in CHUNKS), "odd chunks break ap_gather on HW"
EPS = 1e-7


def build_bass():
    nc = bacc.Bacc()

    x = nc.dram_tensor("x", [ROWS, S], f32, kind="ExternalInput")
    bid_info = nc.dram_tensor("bid_info", [ROWS, 2], i32, kind="ExternalInput")
    out2 = nc.dram_tensor("out2", [ROWS, 2], f32, kind="ExternalOutput")

    # row r = p*64 + k  ->  every DMA contiguous per partition
    x_v = x.rearrange("(p k) s -> p (k s)", p=128)           # [128, 64*512]
    bi_v = bid_info.rearrange("(p k) c -> p (k c)", p=128)   # [128, 128]
    o2_v = out2.rearrange("(p k) c -> p (k c)", p=128)       # [128, 128]

    import contextlib

    # Ring depth per chunk-size class: deep enough to keep the DMA stream
    # gapless, within a ~120KB/partition budget split across classes.
    classes = sorted(set(CHUNKS))
    per_class_kb = 120 // len(classes)
    BUFS = {gc: max(2, min(10, per_class_kb // (2 * gc))) for gc in classes}

    with TileContext(nc) as tc:
        with contextlib.ExitStack() as stack:
            cpool = stack.enter_context(tc.tile_pool(name="const", bufs=1))
            spool = stack.enter_context(tc.tile_pool(name="small", bufs=7))
            pools = {
                gc: stack.enter_context(
                    tc.tile_pool(name=f"big{gc}", bufs=BUFS[gc]))
                for gc in classes
            }

            # ---- bid_info first (tiny; unblocks index prep), then x chunk 0
            bi_all = cpool.tile([128, 2 * K], i32, tag="bi_all")
            nc.sync.dma_start(out=bi_all[:], in_=bi_v)
            g0 = CHUNKS[0]
            xt0 = pools[g0].tile([128, g0 * S], f32, tag=f"xt{g0}")
            w0 = S if SPLIT_HEAD > 0 else g0 * S
            nc.sync.dma_start(out=xt0[:, 0:w0], in_=x_v[:, 0:w0])

            # ---- static constants + index prep, all on Pool (idle until
            # the first gather) so the DVE can start scanning immediately --
            MW = 48 * max(sum(CHUNKS[i:i + EBATCH])
                          for i in range(0, len(CHUNKS), EBATCH))
            rq = cpool.tile([128, MW], i32, tag="rq")          # q%16 per slot
            nc.gpsimd.iota(rq[:], pattern=[[0, MW // 16], [1, 16]], base=0,
                           channel_multiplier=0)
            pp = cpool.tile([128, 1], i32, tag="pp")           # partition idx
            nc.gpsimd.iota(pp[:], pattern=[[1, 1]], base=0,
                           channel_multiplier=1)
            offs32 = cpool.tile([128, 3 * K], i32, tag="offs32")  # glocal*512
            gs = 0
            for gc in CHUNKS:
                nc.gpsimd.iota(offs32[:, 3 * gs:3 * (gs + gc)],
                               pattern=[[512, gc], [0, 3]], base=0,
                               channel_multiplier=0)
                gs += gc

            # Pool handles the casts (proven gpsimd ops); the
            # TensorScalarPtr-encoded arithmetic must stay on DVE (walrus
            # rejects that encoding on Pool).
            pm16 = cpool.tile([128, 1], i32, tag="pm16")
            nc.vector.tensor_scalar(out=pm16[:], in0=pp[:], scalar1=15,
                                    scalar2=None, op0=Alu.bitwise_and)
            pm16f = cpool.tile([128, 1], f32, tag="pm16f")
            nc.gpsimd.tensor_copy(out=pm16f[:], in_=pm16[:])
            rqf = cpool.tile([128, MW], f32, tag="rqf")
            nc.gpsimd.tensor_copy(out=rqf[:], in_=rq[:])
            # one-hot diag mask M[p, s*16+r] = (r == p%16), periodic in s
            m384 = cpool.tile([128, MW], f32, tag="m384")
            nc.vector.tensor_scalar(out=m384[:], in0=rqf[:], scalar1=pm16f[:],
                                    scalar2=None, op0=Alu.is_equal)
            offsf = cpool.tile([128, 3 * K], f32, tag="offsf")
            nc.gpsimd.tensor_copy(out=offsf[:], in_=offs32[:])

            bif = cpool.tile([128, 2 * K], f32, tag="bif")
            nc.gpsimd.tensor_copy(out=bif[:], in_=bi_all[:])
            bif3 = bif[:].rearrange("p (k c) -> p k c", c=2)
            mpf = bif3[:, :, 0]     # [128, 64] market price (strided)
            bidf = bif3[:, :, 1]    # [128, 64] bid

            idxf = cpool.tile([128, 3 * K], f32, tag="idxf")
            idx3 = idxf[:].rearrange("p (k j) -> p k j", j=3)
            # j=0: bid-1, j=1: mp-1, j=2: mp   (clamped at 0; fixups later)
            nc.vector.tensor_scalar(out=idx3[:, :, 0], in0=bidf, scalar1=-1.0,
                                    scalar2=0.0, op0=Alu.add, op1=Alu.max)
            nc.vector.tensor_scalar(out=idx3[:, :, 1], in0=mpf, scalar1=-1.0,
                                    scalar2=0.0, op0=Alu.add, op1=Alu.max)
            nc.gpsimd.tensor_copy(out=idx3[:, :, 2], in_=mpf)
            nc.vector.tensor_tensor(out=idxf[:], in0=idxf[:], in1=offsf[:],
                                    op=Alu.add)
            idx16 = cpool.tile([128, 3 * K], i16, tag="idx16")
            nc.gpsimd.tensor_copy(out=idx16[:], in_=idxf[:])

            # masks + fill constants for the bid==0 / mp==0 fixups
            # (CopyPredicated wants an integer mask dtype)
            mb = cpool.tile([128, K], i32, tag="mb")
            nc.vector.tensor_scalar(out=mb[:], in0=bidf, scalar1=0.0,
                                    scalar2=None, op0=Alu.is_equal)
            mm = cpool.tile([128, K], i32, tag="mm")
            nc.vector.tensor_scalar(out=mm[:], in0=mpf, scalar1=0.0,
                                    scalar2=None, op0=Alu.is_equal)
            ones = cpool.tile([128, K], f32, tag="ones")
            nc.gpsimd.memset(ones[:], 1.0)
            epsc = cpool.tile([128, K], f32, tag="epsc")
            nc.gpsimd.memset(epsc[:], EPS)

            vals = cpool.tile([128, 3 * K], f32, tag="vals")  # [p, k, j]
            ost = cpool.tile([128, 2 * K], f32, tag="ost")    # [p, k, (s,r)]
            ost3 = ost[:].rearrange("p (k c) -> p k c", c=2)
            surv = ost3[:, :, 0]    # strided [128, 64] views
            rate = ost3[:, :, 1]

            v3 = vals[:].rearrange("p (k j) -> p k j", j=3)

            def extract(bgs, bw, gat):
                msk = spool.tile([128, 48 * bw], f32, tag=f"msk{bw}")
                nc.vector.tensor_tensor(out=msk[:], in0=gat[:],
                                        in1=m384[:, 0:48 * bw], op=Alu.mult)
                m3 = msk[:].rearrange("p (s r) -> p s r", r=16)
                nc.vector.tensor_reduce(out=vals[:, 3 * bgs:3 * (bgs + bw)],
                                        in_=m3, axis=Ax.X, op=Alu.add)

            def fix_surv(lo, hi):
                sl = slice(lo, hi)
                # survival = bid==0 ? 1 : cpi[bid-1]
                nc.vector.tensor_copy(out=surv[:, sl], in_=v3[:, sl, 0])
                nc.vector.copy_predicated(out=surv[:, sl], mask=mb[:, sl],
                                          data=ones[:, sl])

            def fix_rate(lo, hi):
                sl = slice(lo, hi)
                # rate = mp==0 ? EPS : cpi[mp-1] - cpi[mp]
                nc.vector.tensor_tensor(out=rate[:, sl], in0=v3[:, sl, 1],
                                        in1=v3[:, sl, 2], op=Alu.subtract)
                nc.vector.copy_predicated(out=rate[:, sl], mask=mm[:, sl],
                                          data=epsc[:, sl])

            def fixups(lo, hi):
                fix_surv(lo, hi)
                fix_rate(lo, hi)

            # ---- main loop over chunks -----------------------------------
            # Gathers land in a shared per-batch tile; one mult+reduce
            # extracts a whole batch (fewer DVE instructions).  Extraction
            # of batch b is issued once batch b+1 is complete, so the
            # in-order DVE meets long-finished Pool gathers.
            batch_w = [sum(CHUNKS[i:i + EBATCH])
                       for i in range(0, len(CHUNKS), EBATCH)]
            pending = []    # (bgs, bw, gat) full batches awaiting extraction
            cur = None      # [bgs, bw, gat, filled] batch being filled
            gs = 0
            n_chunks = len(CHUNKS)
            for ci, gc in enumerate(CHUNKS):
                # The first chunks' DMAs are split per group so their scans
                # (and so the first gathers) chase the DMA stream instead of
                # waiting for the whole chunk to land.
                split = ci < SPLIT_HEAD
                if ci == 0:
                    xt = xt0
                else:
                    xt = pools[gc].tile([128, gc * S], f32, tag=f"xt{gc}")
                    if not split:
                        nc.sync.dma_start(out=xt[:],
                                          in_=x_v[:, gs * S:(gs + gc) * S])
                if split:
                    for g in range(gc):
                        if ci == 0 and g == 0:
                            continue  # xt0's first slice DMA issued up top
                        nc.sync.dma_start(
                            out=xt[:, g * S:(g + 1) * S],
                            in_=x_v[:, (gs + g) * S:(gs + g + 1) * S])

                for g in range(gc):
                    sl = slice(g * S, (g + 1) * S)
                    if split or g == 0:
                        # Tiny read absorbs the HWDGE queue semaphore before
                        # the TensorScalarPtr-encoded scans (that ISA encoding
                        # has too few sync-wait slots to carry it itself).
                        sink = spool.tile([128, 2], f32, tag="sink")
                        nc.vector.tensor_copy(out=sink[:, 0:1],
                                              in_=xt[:, sl.start:sl.start + 1])
                    nc.vector.tensor_tensor_scan(
                        out=xt[:, sl], data0=xt[:, sl], data1=xt[:, sl],
                        initial=1.0, op0=Alu.mult, op1=Alu.bypass)

                if cur is None:
                    bw = batch_w[ci // EBATCH]
                    gat = spool.tile([128, 48 * bw], f32, tag=f"gatb{bw}")
                    cur = [gs, bw, gat, 0]
                off = cur[3]
                nc.gpsimd.ap_gather(
                    out_ap=cur[2][:, 48 * off:48 * (off + gc)], in_ap=xt[:],
                    idxs_ap=idx16[:, 3 * gs:3 * (gs + gc)],
                    channels=128, num_elems=gc * S, d=1, num_idxs=48 * gc)
                cur[3] += gc

                if cur[3] == cur[1]:
                    pending.append((cur[0], cur[1], cur[2]))
                    cur = None
                    while len(pending) > DEFER:
                        extract(*pending.pop(0))
                if ci == n_chunks - 1:
                    # head fixups run on DVE while the last gathers run
                    while len(pending) > 1:
                        extract(*pending.pop(0))
                    fixups(0, pending[0][0])
                gs += gc

            tail_lo = pending[0][0]
            extract(*pending.pop(0))
            fix_surv(tail_lo, K)
            fix_rate(tail_lo, K)
            nc.sync.dma_start(out=o2_v, in_=ost[:])
    nc.finalize()
    return nc


_NC_CACHE = None


def _get_nc():
    global _NC_CACHE
    if _NC_CACHE is None:
        _NC_CACHE = build_bass()
    return _NC_CACHE


def kernel(x, bid_info):
    x = np.ascontiguousarray(np.asarray(x, dtype=np.float32))
    bid_info = np.ascontiguousarray(np.asarray(bid_info, dtype=np.int32))
    assert x.shape == (B, S) and bid_info.shape == (B, 2)

    nc = _get_nc()
    in_maps = [
        {
            "x": x[c * ROWS:(c + 1) * ROWS],
            "bid_info": bid_info[c * ROWS:(c + 1) * ROWS],
        }
        for c in range(N_CORES)
    ]
    res = run_bass_kernel_spmd(nc, in_maps, core_ids=list(range(N_CORES)))
    out2 = np.concatenate([r["out2"] for r in res.results], axis=0)
    return np.ascontiguousarray(out2[:, 0:1]), np.ascontiguousarray(out2[:, 1:2])


# revision 49
# speedup vs baseline: 1.0026x; 1.0016x over previous
"""Trainium2 Bass kernel for nn_BidPrefix (segment_reduce).

Reference semantics, per row r (B=65536 rows, S=512 cols):
    cp[k]    = prod(x[r, 0:k])                  (exclusive prefix product)
    survival = cp[bid]
    rate     = cp[mp] - cp[mp+1], or EPS when mp == 0
returned as (survival [B,1] f32, rate_last [B,1] f32).

Design: exact fp32 inclusive cumprod via DVE tensor_tensor_scan
(op0=mult, op1=bypass; one 512-long recurrence per row-group, written
in place over the x tile), then a per-row 3-element extraction with one
GPSIMD ap_gather per chunk:
    survival = cpi[bid-1]   (bid==0 -> 1, fixed up)
    g1       = cpi[mp-1]    (mp==0 handled by the EPS fixup)
    g2       = cpi[mp]
    rate     = mp==0 ? EPS : g1 - g2
ap_gather applies, for each 16-partition GPSIMD core, the index list
stored across its 16 partitions (slot s of partition p = flat position
q = s*16 + p%16) to ALL 16 channels; row p's own values land at
out[p, s*16 + p%16] and are pulled out with a static one-hot mask
(mult + segmented reduce).  Indices are pre-offset by glocal*512 so a
single gather covers a whole multi-group chunk.

Schedule: the three ~47us engine streams (DMA-in, DVE scans+extracts,
Pool gathers) are software-pipelined: a chunk's extract is deferred
DEFER chunks so the in-order DVE never stalls on the Pool gather it
consumes.  Chunk sizes taper (4..4,2..2 groups) to shorten pipeline
fill and drain; index prep casts run on the (otherwise idle
until the first gather) Pool engine; fixups are predicated copies,
split so only the last chunk's sliver trails the final gather; both
outputs interleave into one DRAM tensor so the tail pays a single
DMA chain.

Row mapping r = p*64 + k keeps every DMA contiguous per partition:
x chunks 4-16KB, bid_info 512B, output 512B.

Sharding: pure data parallel over the batch axis, B/8 = 8192 rows per
NeuronCore, same NEFF on all 8 cores (SPMD), outputs concatenated.
"""

import numpy as np

import concourse.bacc as bacc
import concourse.mybir as mybir
from concourse.tile import TileContext
from concourse.bass_utils import run_bass_kernel_spmd

f32 = mybir.dt.float32
i32 = mybir.dt.int32
i16 = mybir.dt.int16
Alu = mybir.AluOpType
Ax = mybir.AxisListType

N_CORES = 8
B, S = 65536, 512
ROWS = B // N_CORES          # 8192 rows per core
K = ROWS // 128              # 64 row-groups per partition
# Chunk sizes must be EVEN: a chunk's int16 index slice is 3*gc slots per
# partition, and the hardware ap_gather ucode needs 4-byte-aligned index
# slices (odd gc -> 18-byte offsets -> silently wrong gathers on HW, even
# though the interpreter/cost model accept it).
CHUNKS = [4] * 10 + [2] * 12                # groups per chunk, sum = 64
EBATCH = 1                                 # chunks per extract batch
DEFER = 2                                  # batches between gather and extract
SPLIT_HEAD = 0                             # leading chunks with per-group DMA
assert sum(CHUNKS) == K
assert all(gc % 2 == 0 for gc in CHUNKS), "odd chunks break ap_gather on HW"
EPS = 1e-7


def build_bass():
    nc = bacc.Bacc()

    x = nc.dram_tensor("x", [ROWS, S], f32, kind="ExternalInput")
    bid_info = nc.dram_tensor("bid_info", [ROWS, 2], i32, kind="ExternalInput")
    out2 = nc.dram_tensor("out2", [ROWS, 2], f32, kind="ExternalOutput")

    # row r = p*64 + k  ->  every DMA contiguous per partition
    x_v = x.rearrange("(p k) s -> p (k s)", p=128)           # [128, 64*512]
    bi_v = bid_info.rearrange("(p k) c -> p (k c)", p=128)   # [128, 128]
    o2_v = out2.rearrange("(p k) c -> p (k c)", p=128)       # [128, 128]

    import contextlib

    # Ring depth per chunk-size class: deep enough to keep the DMA stream
    # gapless, within a ~120KB/partition budget split across classes.
    classes = sorted(set(CHUNKS))
    per_class_kb = 120 // len(classes)
    BUFS = {gc: max(2, min(10, per_class_kb // (2 * gc))) for gc in classes}

    with TileContext(nc) as tc:
        with contextlib.ExitStack() as stack:
            cpool = stack.enter_context(tc.tile_pool(name="const", bufs=1))
            spool = stack.enter_context(tc.tile_pool(name="small", bufs=7))
            pools = {
                gc: stack.enter_context(
                    tc.tile_pool(name=f"big{gc}", bufs=BUFS[gc]))
                for gc in classes
            }

            # ---- bid_info first (tiny; unblocks index prep), then x chunk 0
            bi_all = cpool.tile([128, 2 * K], i32, tag="bi_all")
            nc.sync.dma_start(out=bi_all[:], in_=bi_v)
            g0 = CHUNKS[0]
            xt0 = pools[g0].tile([128, g0 * S], f32, tag=f"xt{g0}")
            w0 = S if SPLIT_HEAD > 0 else g0 * S
            nc.sync.dma_start(out=xt0[:, 0:w0], in_=x_v[:, 0:w0])

            # ---- static constants + index prep, all on Pool (idle until
            # the first gather) so the DVE can start scanning immediately --
            MW = 48 * max(sum(CHUNKS[i:i + EBATCH])
                          for i in range(0, len(CHUNKS), EBATCH))
            rq = cpool.tile([128, MW], i32, tag="rq")          # q%16 per slot
            nc.gpsimd.iota(rq[:], pattern=[[0, MW // 16], [1, 16]], base=0,
                           channel_multiplier=0)
            pp = cpool.tile([128, 1], i32, tag="pp")           # partition idx
            nc.gpsimd.iota(pp[:], pattern=[[1, 1]], base=0,
                           channel_multiplier=1)
            offs32 = cpool.tile([128, 3 * K], i32, tag="offs32")  # glocal*512
            gs = 0
            for gc in CHUNKS:
                nc.gpsimd.iota(offs32[:, 3 * gs:3 * (gs + gc)],
                               pattern=[[512, gc], [0, 3]], base=0,
                               channel_multiplier=0)
                gs += gc

            # Pool handles the casts (proven gpsimd ops); the
            # TensorScalarPtr-encoded arithmetic must stay on DVE (walrus
            # rejects that encoding on Pool).
            pm16 = cpool.tile([128, 1], i32, tag="pm16")
            nc.vector.tensor_scalar(out=pm16[:], in0=pp[:], scalar1=15,
                                    scalar2=None, op0=Alu.bitwise_and)
            pm16f = cpool.tile([128, 1], f32, tag="pm16f")
            nc.gpsimd.tensor_copy(out=pm16f[:], in_=pm16[:])
            rqf = cpool.tile([128, MW], f32, tag="rqf")
            nc.gpsimd.tensor_copy(out=rqf[:], in_=rq[:])
            # one-hot diag mask M[p, s*16+r] = (r == p%16), periodic in s
            m384 = cpool.tile([128, MW], f32, tag="m384")
            nc.vector.tensor_scalar(out=m384[:], in0=rqf[:], scalar1=pm16f[:],
                                    scalar2=None, op0=Alu.is_equal)
            offsf = cpool.tile([128, 3 * K], f32, tag="offsf")
            nc.gpsimd.tensor_copy(out=offsf[:], in_=offs32[:])

            bif = cpool.tile([128, 2 * K], f32, tag="bif")
            nc.gpsimd.tensor_copy(out=bif[:], in_=bi_all[:])
            bif3 = bif[:].rearrange("p (k c) -> p k c", c=2)
            mpf = bif3[:, :, 0]     # [128, 64] market price (strided)
            bidf = bif3[:, :, 1]    # [128, 64] bid

            idxf = cpool.tile([128, 3 * K], f32, tag="idxf")
            idx3 = idxf[:].rearrange("p (k j) -> p k j", j=3)
            # j=0: bid-1, j=1: mp-1, j=2: mp   (clamped at 0; fixups later)
            nc.vector.tensor_scalar(out=idx3[:, :, 0], in0=bidf, scalar1=-1.0,
                                    scalar2=0.0, op0=Alu.add, op1=Alu.max)
            nc.vector.tensor_scalar(out=idx3[:, :, 1], in0=mpf, scalar1=-1.0,
                                    scalar2=0.0, op0=Alu.add, op1=Alu.max)
            nc.gpsimd.tensor_copy(out=idx3[:, :, 2], in_=mpf)
            nc.vector.tensor_tensor(out=idxf[:], in0=idxf[:], in1=offsf[:],
                                    op=Alu.add)
            idx16 = cpool.tile([128, 3 * K], i16, tag="idx16")
            nc.gpsimd.tensor_copy(out=idx16[:], in_=idxf[:])

            # masks + fill constants for the bid==0 / mp==0 fixups
            # (CopyPredicated wants an integer mask dtype)
            mb = cpool.tile([128, K], i32, tag="mb")
            nc.vector.tensor_scalar(out=mb[:], in0=bidf, scalar1=0.0,
                                    scalar2=None, op0=Alu.is_equal)
            mm = cpool.tile([128, K], i32, tag="mm")
            nc.vector.tensor_scalar(out=mm[:], in0=mpf, scalar1=0.0,
                                    scalar2=None, op0=Alu.is_equal)
            ones = cpool.tile([128, K], f32, tag="ones")
            nc.gpsimd.memset(ones[:], 1.0)
            epsc = cpool.tile([128, K], f32, tag="epsc")
            nc.gpsimd.memset(epsc[:], EPS)

            vals = cpool.tile([128, 3 * K], f32, tag="vals")  # [p, k, j]
            ost = cpool.tile([128, 2 * K], f32, tag="ost")    # [p, k, (s,r)]
            ost3 = ost[:].rearrange("p (k c) -> p k c", c=2)
            surv = ost3[:, :, 0]    # strided [128, 64] views
            rate = ost3[:, :, 1]

            v3 = vals[:].rearrange("p (k j) -> p k j", j=3)

            def extract(bgs, bw, gat):
                msk = spool.tile([128, 48 * bw], f32, tag=f"msk{bw}")
                nc.vector.tensor_tensor(out=msk[:], in0=gat[:],
                                        in1=m384[:, 0:48 * bw], op=Alu.mult)
                m3 = msk[:].rearrange("p (s r) -> p s r", r=16)
                nc.vector.tensor_reduce(out=vals[:, 3 * bgs:3 * (bgs + bw)],
                                        in_=m3, axis=Ax.X, op=Alu.add)

            def fix_surv(lo, hi):
                sl = slice(lo, hi)
                # survival = bid==0 ? 1 : cpi[bid-1]
                nc.vector.tensor_copy(out=surv[:, sl], in_=v3[:, sl, 0])
                nc.vector.copy_predicated(out=surv[:, sl], mask=mb[:, sl],
                                          data=ones[:, sl])

            def fix_rate(lo, hi):
                sl = slice(lo, hi)
                # rate = mp==0 ? EPS : cpi[mp-1] - cpi[mp]
                nc.vector.tensor_tensor(out=rate[:, sl], in0=v3[:, sl, 1],
                                        in1=v3[:, sl, 2], op=Alu.subtract)
                nc.vector.copy_predicated(out=rate[:, sl], mask=mm[:, sl],
                                          data=epsc[:, sl])

            def fixups(lo, hi):
                fix_surv(lo, hi)
                fix_rate(lo, hi)

            # ---- main loop over chunks -----------------------------------
            # Gathers land in a shared per-batch tile; one mult+reduce
            # extracts a whole batch (fewer DVE instructions).  Extraction
            # of batch b is issued once batch b+1 is complete, so the
            # in-order DVE meets long-finished Pool gathers.
            batch_w = [sum(CHUNKS[i:i + EBATCH])
                       for i in range(0, len(CHUNKS), EBATCH)]
            pending = []    # (bgs, bw, gat) full batches awaiting extraction
            cur = None      # [bgs, bw, gat, filled] batch being filled
            gs = 0
            n_chunks = len(CHUNKS)
            for ci, gc in enumerate(CHUNKS):
                # The first chunks' DMAs are split per group so their scans
                # (and so the first gathers) chase the DMA stream instead of
                # waiting for the whole chunk to land.
                split = ci < SPLIT_HEAD
                if ci == 0:
                    xt = xt0
                else:
                    xt = pools[gc].tile([128, gc * S], f32, tag=f"xt{gc}")
                    if not split:
                        nc.sync.dma_start(out=xt[:],
                                          in_=x_v[:, gs * S:(gs + gc) * S])
                if split:
                    for g in range(gc):
                        if ci == 0 and g == 0:
                            continue  # xt0's first slice DMA issued up top
                        nc.sync.dma_start(
                            out=xt[:, g * S:(g + 1) * S],
                            in_=x_v[:, (gs + g) * S:(gs + g + 1) * S])

                for g in range(gc):
                    sl = slice(g * S, (g + 1) * S)
                    if split or g == 0:
                        # Tiny read absorbs the HWDGE queue semaphore before
                        # the TensorScalarPtr-encoded scans (that ISA encoding
                        # has too few sync-wait slots to carry it itself).
                        sink = spool.tile([128, 2], f32, tag="sink")
                        nc.vector.tensor_copy(out=sink[:, 0:1],
                                              in_=xt[:, sl.start:sl.start + 1])
                    nc.vector.tensor_tensor_scan(
                        out=xt[:, sl], data0=xt[:, sl], data1=xt[:, sl],
                        initial=1.0, op0=Alu.mult, op1=Alu.bypass)

                if cur is None:
                    bw = batch_w[ci // EBATCH]
                    gat = spool.tile([128, 48 * bw], f32, tag=f"gatb{bw}")
                    cur = [gs, bw, gat, 0]
                off = cur[3]
                nc.gpsimd.ap_gather(
                    out_ap=cur[2][:, 48 * off:48 * (off + gc)], in_ap=xt[:],
                    idxs_ap=idx16[:, 3 * gs:3 * (gs + gc)],
                    channels=128, num_elems=gc * S, d=1, num_idxs=48 * gc)
                cur[3] += gc

                if cur[3] == cur[1]:
                    pending.append((cur[0], cur[1], cur[2]))
                    cur = None
                    while len(pending) > DEFER:
                        extract(*pending.pop(0))
                if ci == n_chunks - 1:
                    # head fixups run on DVE while the last gathers run
                    while len(pending) > 1:
                        extract(*pending.pop(0))
                    fixups(0, pending[0][0])
                gs += gc

            tail_lo = pending[0][0]
            extract(*pending.pop(0))
            fix_surv(tail_lo, K)
            fix_rate(tail_lo, K)
            nc.sync.dma_start(out=o2_v, in_=ost[:])
    nc.finalize()
    return nc


_NC_CACHE = None


def _get_nc():
    global _NC_CACHE
    if _NC_CACHE is None:
        _NC_CACHE = build_bass()
    return _NC_CACHE


def kernel(x, bid_info):
    x = np.ascontiguousarray(np.asarray(x, dtype=np.float32))
    bid_info = np.ascontiguousarray(np.asarray(bid_info, dtype=np.int32))
    assert x.shape == (B, S) and bid_info.shape == (B, 2)

    nc = _get_nc()
    in_maps = [
        {
            "x": x[c * ROWS:(c + 1) * ROWS],
            "bid_info": bid_info[c * ROWS:(c + 1) * ROWS],
        }
        for c in range(N_CORES)
    ]
    res = run_bass_kernel_spmd(nc, in_maps, core_ids=list(range(N_CORES)))
    out2 = np.concatenate([r["out2"] for r in res.results], axis=0)
    return np.ascontiguousarray(out2[:, 0:1]), np.ascontiguousarray(out2[:, 1:2])
